# revision 35
# baseline (speedup 1.0000x reference)
"""Multi-head attention (B=2, T=2048, C=2048, H=16, causal, interleaved RoPE)
as a Bass/Tile kernel on 8 Trainium2 NeuronCores.

Sharding: core c handles batch b = c // 4 and heads 4*(c % 4) .. 4*(c % 4)+4.
Each core computes QKV for its heads, RoPE, causal attention, and the partial
output projection (row-parallel W_proj). Host sums the 4 partials per batch
and adds b_proj.

Device layouts (per core):
  - q, k are produced transposed [D=128(part), T] straight out of the QKV
    matmul (lhsT = W block, rhs = x^T).  The head dim is host-permuted to
    [even dims; odd dims] so interleaved RoPE is pure within-half DVE math
    (plus one small intra-SBUF DMA for the half swap).
  - v is produced natural [T(part), D] (lhsT = x^T block, rhs = W_v).
  - scores are computed transposed [Tk(part), Tq] so exp(scores)^T directly
    feeds the PV matmul as the moving operand.  Softmax denominators: DVE
    accumulates per-(partition, column) prob sums across k-blocks (hidden
    under PE), then per head PE reduces them to one row (ones-column
    matmul), DVE takes the reciprocal, PE broadcasts it back to all 128
    partitions (ones-row matmul), and one DVE multiply normalizes the PV
    accumulator on eviction.  The normalization tail is emitted one head
    late so PE is never parked on DVE.
  - all input DMAs are single multi-dim-AP transfers (one per tensor /
    xt chunk) spread across the SP and Activation HWDGE queues; output
    partials are written as bf16, one DMA per 128-token block.
Dtypes: all matmuls bf16 (except fp32r ones-reductions/broadcasts);
accumulation fp32; output partials bf16, summed on host in fp32.
"""

import math

import numpy as np

P = 128  # partitions
B, T, C, H = 2, 2048, 2048, 16
D = C // H  # 128
NCORES = 8
GROUPS = 4  # head-groups per batch
HPC = H // GROUPS  # heads per core = 4
ROPE_BASE = 10000.0
NEG = -1e9

_CACHE = {}


FULL_PARTS = ("dma", "p1", "p2", "p3", "outdma")


def build_nc(T=T, C=C, HPC=HPC, TCX=512, TC=512, reps=1, parts=FULL_PARTS):
    """Build + compile the per-core Bass program (SPMD: same NEFF, 8 cores).

    reps > 1 replicates the whole computation on-device (for benchmarking:
    dispatch overhead cancels between reps=1 and reps=k timings).
    parts subsets the body for timing attribution probes (default: full
    kernel; correctness requires the default).
    """
    import concourse.bacc as bacc
    import concourse.mybir as mybir
    import concourse.tile as tile

    dt = mybir.dt
    Act = mybir.ActivationFunctionType
    CS = C // P  # contraction slabs
    TB = T // P  # token blocks
    VC = HPC * D  # v columns per core (= 512 at full size)
    KBC = TC // P  # k-blocks per Tq chunk
    scale = 1.0 / math.sqrt(D)

    nc = bacc.Bacc("TRN2", target_bir_lowering=False, debug=False)
    with tile.TileContext(nc) as tc:
        with tc.tile_pool(name="dram", bufs=1, space="DRAM") as dram:

            def din(name, shape, dtype):
                return dram.tile(
                    shape, dtype, kind="ExternalInput", name=name, uniquify=False
                )

            xT = din("xT", [C, T], dt.bfloat16)  # x[b].T
            Wqk = din("Wqk", [C, 2 * VC], dt.bfloat16)  # [q|k], permuted
            Wv = din("Wv", [C, VC], dt.bfloat16)
            bqk = din("bqk", [P, 2 * HPC], dt.float32)  # per-dim bias cols (q,k)
            bv = din("bv", [1, VC], dt.float32r)
            trigA = din("trigA", [P, T], dt.float32)  # [cos; cos]
            trigB = din("trigB", [P, T], dt.float32)  # [-sin; sin]
            tri = din("tri", [P, P], dt.float32)  # causal triangle mask
            Wp = din("Wp", [VC, C], dt.bfloat16)  # W_proj rows for this core
            onesr = din("onesr", [1, P], dt.float32r)
            onesc = din("onesc", [P, 1], dt.float32r)
            out = dram.tile(
                [T, C], dt.bfloat16, kind="ExternalOutput", name="out", uniquify=False
            )

            xT_r = xT.rearrange("(s p) t -> p s t", p=P)
            Wqk_r = Wqk.rearrange("(s p) n -> p s n", p=P)
            Wv_r = Wv.rearrange("(s p) n -> p s n", p=P)
            Wp_r = Wp.rearrange("(s p) n -> p s n", p=P)

            if "hoist" in parts:
                # DIAGNOSTIC ONLY: load all weights once, outside the rep
                # loop, to size the per-rep weight-reload bubble.
                with tc.tile_pool(name="hoistw", bufs=1) as hw_pool:
                    pre = {
                        nm: hw_pool.tile(shape, dty, name="hoist_" + nm)
                        for nm, shape, dty in (
                            ("w_sb", [P, CS, 2 * VC], dt.bfloat16),
                            ("wv_sb", [P, CS, VC], dt.bfloat16),
                            ("trigA_sb", [P, T], dt.float32),
                            ("trigB_sb", [P, T], dt.float32),
                            ("bqk_sb", [P, 2 * HPC], dt.float32),
                            ("bv_sb", [1, VC], dt.float32r),
                            ("wp_sb", [P, HPC, C], dt.bfloat16),
                        )
                    }
                    nc.scalar.dma_start(out=pre["w_sb"][:], in_=Wqk_r[:])
                    nc.scalar.dma_start(out=pre["wv_sb"][:], in_=Wv_r[:])
                    nc.scalar.dma_start(out=pre["trigA_sb"], in_=trigA[:])
                    nc.scalar.dma_start(out=pre["trigB_sb"], in_=trigB[:])
                    nc.scalar.dma_start(out=pre["bqk_sb"], in_=bqk[:])
                    nc.scalar.dma_start(out=pre["bv_sb"], in_=bv[:])
                    nc.scalar.dma_start(out=pre["wp_sb"], in_=Wp_r)
                    for rep in range(reps):
                        _emit_body(
                            nc, tc, dt, Act, rep,
                            xT_r, Wqk_r, Wv_r, Wp_r, bqk, bv, trigA, trigB,
                            tri, onesr, onesc, out,
                            T, C, HPC, TCX, TC, CS, TB, VC, KBC, scale, parts,
                            pre=pre,
                        )
            else:
                for rep in range(reps):
                    _emit_body(
                        nc, tc, dt, Act, rep,
                        xT_r, Wqk_r, Wv_r, Wp_r, bqk, bv, trigA, trigB, tri,
                        onesr, onesc, out,
                        T, C, HPC, TCX, TC, CS, TB, VC, KBC, scale, parts,
                    )
    nc.compile()
    return nc


def _emit_body(
    nc, tc, dt, Act, rep,
    xT_r, Wqk_r, Wv_r, Wp_r, bqk, bv, trigA, trigB, tri, onesr, onesc, out,
    T, C, HPC, TCX, TC, CS, TB, VC, KBC, scale, parts=FULL_PARTS, pre=None,
):
    import concourse.bass_isa as bass_isa
    HD = D // 2
    dma = "dma" in parts
    sfx = f"_{rep}"
    with tc.tile_pool(name="persist" + sfx, bufs=1) as persist:
        qrot = persist.tile([P, HPC, T], dt.bfloat16)
        krot = persist.tile([P, HPC, T], dt.bfloat16)
        v_sb = persist.tile([P, TB, VC], dt.bfloat16)
        ones_row = persist.tile([1, P], dt.float32r)
        tri_sb = persist.tile([P, P], dt.float32)
        ones_col = persist.tile([P, 1], dt.float32r)
        ones_bcr = persist.tile([1, P], dt.float32r)
        nc.sync.dma_start(out=ones_col, in_=onesc[:])
        nc.sync.dma_start(out=ones_bcr, in_=onesr[:])
        if dma:
            nc.sync.dma_start(out=ones_row, in_=onesr[:])
            nc.sync.dma_start(out=tri_sb, in_=tri[:])
        else:
            nc.gpsimd.memset(ones_row[:], 1.0)
            nc.gpsimd.memset(tri_sb[:], 0.0)

        # ---------------- Phase 1: QKV + RoPE ----------------
        with tc.tile_pool(name="p1w" + sfx, bufs=1) as p1w, tc.tile_pool(
            name="p1xt" + sfx, bufs=(2 if pre is not None else 3)
        ) as p1xt, tc.tile_pool(name="p1st" + sfx, bufs=2) as p1st, tc.tile_pool(
            name="p1t1" + sfx, bufs=3
        ) as p1t1, tc.tile_pool(
            name="p1ps" + sfx, bufs=6, space="PSUM"
        ) as p1ps:
            # DMA emission ordered by first-need time: xt0 (t=0), q/k weights
            # (t=0), trig+biases (first rope, ~10us), v weights (~40us),
            # xt1 (~47us).
            xt_pre = []
            xt_c = p1xt.tile([P, CS, TCX], dt.bfloat16, tag="xt_sb")
            if dma:
                nc.sync.dma_start(out=xt_c[:], in_=xT_r[:, :, 0:TCX])
            else:
                nc.gpsimd.memset(xt_c[:], 0.0)
            xt_pre.append(xt_c)
            if pre is not None:
                w_sb = pre["w_sb"]
                trigA_sb = pre["trigA_sb"]
                trigB_sb = pre["trigB_sb"]
                bqk_sb = pre["bqk_sb"]
                bv_sb = pre["bv_sb"]
                wv_sb = pre["wv_sb"]
            else:
                w_sb = p1w.tile([P, CS, 2 * VC], dt.bfloat16)
                if dma:
                    nc.scalar.dma_start(out=w_sb[:], in_=Wqk_r[:])
                trigA_sb = p1w.tile([P, T], dt.float32)
                trigB_sb = p1w.tile([P, T], dt.float32)
                bqk_sb = p1w.tile([P, 2 * HPC], dt.float32)
                bv_sb = p1w.tile([1, VC], dt.float32r)
                wv_sb = p1w.tile([P, CS, VC], dt.bfloat16)
                if dma:
                    nc.scalar.dma_start(out=trigA_sb, in_=trigA[:])
                    nc.scalar.dma_start(out=trigB_sb, in_=trigB[:])
                    nc.scalar.dma_start(out=bqk_sb, in_=bqk[:])
                    nc.scalar.dma_start(out=bv_sb, in_=bv[:])
                    nc.scalar.dma_start(out=wv_sb[:], in_=Wv_r[:])
                else:
                    nc.gpsimd.memset(w_sb[:], 0.0)
                    nc.gpsimd.memset(trigA_sb[:], 0.0)
                    nc.gpsimd.memset(trigB_sb[:], 0.0)
                    nc.gpsimd.memset(bqk_sb[:], 0.0)
                    nc.gpsimd.memset(bv_sb[:], 0.0)
                    nc.gpsimd.memset(wv_sb[:], 0.0)
            xt_c = p1xt.tile([P, CS, TCX], dt.bfloat16, tag="xt_sb")
            if dma:
                nc.sync.dma_start(out=xt_c[:], in_=xT_r[:, :, TCX : 2 * TCX])
            else:
                nc.gpsimd.memset(xt_c[:], 0.0)
            xt_pre.append(xt_c)

            NQK = 2 * HPC  # 8 (q|k, head) results per chunk
            for tx in range(T // TCX):
                tsl = slice(tx * TCX, (tx + 1) * TCX)
                if tx < 2:
                    xt_sb = xt_pre[tx]
                else:
                    xt_sb = p1xt.tile([P, CS, TCX], dt.bfloat16, tag="xt_sb")
                    if dma:
                        nc.sync.dma_start(out=xt_sb[:], in_=xT_r[:, :, tsl])
                    else:
                        nc.gpsimd.memset(xt_sb[:], 0.0)
                if "p1" not in parts:
                    continue
                # q, k transposed per head: [D, TCX]; all 8 head results land
                # in one [P, 8, TCX] tile so the RoPE half-swap is 2 DMAs.
                st = p1st.tile([P, NQK, TCX], dt.bfloat16, tag="st")
                sw = p1st.tile([P, NQK, TCX], dt.bfloat16, tag="sw")
                for qk in range(2):
                    for h in range(HPC):
                        idx = qk * HPC + h
                        col = idx * D
                        ps = p1ps.tile([P, TCX], dt.float32)
                        for s in range(CS):
                            nc.tensor.matmul(
                                ps[:],
                                w_sb[:, s, col : col + D],
                                xt_sb[:, s, :],
                                start=(s == 0),
                                stop=(s == CS - 1),
                            )
                        nc.scalar.activation(
                            st[:, idx], ps[:], Act.Identity,
                            bias=bqk_sb[:, idx : idx + 1],
                        )
                # RoPE: rot = st*[cos;cos] + swap(st)*[-sin;sin]
                nc.sync.dma_start(out=sw[0:HD], in_=st[HD : 2 * HD])
                nc.sync.dma_start(out=sw[HD : 2 * HD], in_=st[0:HD])
                for qk in range(2):
                    dest = qrot if qk == 0 else krot
                    for h in range(HPC):
                        idx = qk * HPC + h
                        t1 = p1t1.tile([P, TCX], dt.float32)
                        nc.vector.tensor_mul(t1[:], st[:, idx], trigA_sb[:, tsl])
                        nc.vector.tensor_mul(sw[:, idx], sw[:, idx], trigB_sb[:, tsl])
                        nc.vector.tensor_add(dest[:, h, tsl], t1[:], sw[:, idx])
                # v natural rows
                for tb in range(TCX // P):
                    kb = tx * (TCX // P) + tb
                    ps = p1ps.tile([P, VC], dt.float32)
                    for s in range(CS):
                        nc.tensor.matmul(
                            ps[:],
                            xt_sb[:, s, tb * P : (tb + 1) * P],
                            wv_sb[:, s, :],
                            start=(s == 0),
                            stop=False,
                        )
                    nc.tensor.matmul(
                        ps[:], ones_row[:], bv_sb[:], start=False, stop=True
                    )
                    nc.vector.tensor_copy(out=v_sb[:, kb, :], in_=ps[:])

        # ---------------- Phase 2: causal attention ----------------
        with tc.tile_pool(name="p2a" + sfx, bufs=1) as p2a:
            attnT = p2a.tile([P, HPC, T], dt.bfloat16)
            if pre is not None:
                wp_sb = pre["wp_sb"]
            else:
                wp_sb = p2a.tile([P, HPC, C], dt.bfloat16)
                if dma:
                    nc.sync.dma_start(out=wp_sb, in_=Wp_r)
                elif "p3" in parts:
                    nc.gpsimd.memset(wp_sb[:], 0.0)

            with tc.tile_pool(name="p2probs" + sfx, bufs=6) as p2probs, tc.tile_pool(
                name="p2b" + sfx, bufs=3
            ) as p2b, tc.tile_pool(
                name="p2sc" + sfx, bufs=3, space="PSUM"
            ) as p2sc, tc.tile_pool(
                name="p2acc" + sfx, bufs=2, space="PSUM"
            ) as p2acc, tc.tile_pool(
                name="p2l" + sfx, bufs=2, space="PSUM"
            ) as p2l, tc.tile_pool(
                name="p2bc" + sfx, bufs=1, space="PSUM"
            ) as p2bc:

                def norm_tail(h_, tq_, a_ps_, sum_sb_):
                    # PE-reduce the DVE-accumulated sums to one row, take the
                    # reciprocal, PE-broadcast to all partitions, then
                    # normalize-evict from psum.  Emitted one head late so PE
                    # is never parked on DVE.
                    qsl_ = slice(tq_ * TC, (tq_ + 1) * TC)
                    l_ps = p2l.tile([1, TC], dt.float32)
                    nc.tensor.matmul(
                        l_ps[:], ones_col[:], sum_sb_[:], start=True, stop=True
                    )
                    r_sb = p2b.tile([1, TC], dt.float32r)
                    with nc.allow_low_precision(
                        reason="float32r == fp32 storage; relaxed matmul ok"
                    ):
                        nc.vector.reciprocal(r_sb[:], l_ps[:])
                    rec_bc = p2bc.tile([P, TC], dt.float32)
                    nc.tensor.matmul(
                        rec_bc[:], ones_bcr[:], r_sb[:], start=True, stop=True
                    )
                    rec_sb = p2b.tile([P, TC], dt.float32, tag="rec_sb")
                    nc.scalar.copy(rec_sb[:], rec_bc[:])
                    nc.vector.tensor_mul(
                        attnT[:, h_, qsl_], a_ps_[:], rec_sb[:]
                    )

                pending = None
                for tq in range(T // TC if "p2" in parts else 0):
                    for h in range(HPC):
                        nkb = (tq + 1) * KBC
                        a_ps = p2acc.tile([P, TC], dt.float32)
                        sum_sb = p2b.tile([P, TC], dt.float32r, tag="sum_sb")
                        for kb in range(nkb):
                            # diagonal blocks: columns < col0 are fully masked
                            # -> skip them in scores/exp/sums/PV and mask only
                            # the [P, P] triangle.
                            j = kb - tq * KBC
                            col0 = max(j, 0) * P
                            csl3 = slice(col0, TC)
                            s_ps = p2sc.tile([P, TC], dt.float32)
                            nc.tensor.matmul(
                                s_ps[:, csl3],
                                krot[:, h, kb * P : (kb + 1) * P],
                                qrot[:, h, tq * TC + col0 : (tq + 1) * TC],
                                start=True,
                                stop=True,
                            )
                            if j >= 0:
                                nc.vector.tensor_add(
                                    s_ps[:, col0 : col0 + P],
                                    s_ps[:, col0 : col0 + P],
                                    tri_sb[:],
                                )
                            pt = p2probs.tile([P, TC], dt.bfloat16)
                            nc.scalar.activation(
                                pt[:, csl3], s_ps[:, csl3], Act.Exp, scale=scale
                            )
                            # running per-(partition, column) prob sums on DVE
                            # (hidden under PE); PE reduces them once per head
                            # in norm_tail.
                            with nc.allow_low_precision(
                                reason="float32r == fp32 storage"
                            ):
                                if kb == 0:
                                    nc.vector.tensor_copy(
                                        out=sum_sb[:], in_=pt[:]
                                    )
                                else:
                                    nc.vector.tensor_add(
                                        sum_sb[:, csl3],
                                        sum_sb[:, csl3],
                                        pt[:, csl3],
                                    )
                            nc.tensor.matmul(
                                a_ps[:, csl3],
                                v_sb[:, kb, h * D : (h + 1) * D],
                                pt[:, csl3],
                                start=(kb == 0),
                                stop=(kb == nkb - 1),
                                skip_group_check=True,
                            )
                            if kb == 1 and pending is not None:
                                norm_tail(*pending)
                                pending = None
                        pending = (h, tq, a_ps, sum_sb)
                if pending is not None:
                    norm_tail(*pending)

            # ---------------- Phase 3: output projection ----------------
            with tc.tile_pool(name="p3o" + sfx, bufs=3) as p3o, tc.tile_pool(
                name="p3ps" + sfx, bufs=6, space="PSUM"
            ) as p3ps:
                NCH = 512
                for tb in range(TB if "p3" in parts else 0):
                    o_sb = p3o.tile([P, C], dt.bfloat16)
                    for ncol in range(C // NCH):
                        csl = slice(ncol * NCH, (ncol + 1) * NCH)
                        ps = p3ps.tile([P, NCH], dt.float32)
                        for j in range(HPC):
                            nc.tensor.matmul(
                                ps[:],
                                attnT[:, j, tb * P : (tb + 1) * P],
                                wp_sb[:, j, csl],
                                start=(j == 0),
                                stop=(j == HPC - 1),
                            )
                        if ncol % 2 == 0:
                            nc.scalar.copy(o_sb[:, csl], ps[:])
                        else:
                            nc.vector.tensor_copy(out=o_sb[:, csl], in_=ps[:])
                    if "outdma" in parts:
                        nc.scalar.dma_start(
                            out=out[tb * P : (tb + 1) * P, :], in_=o_sb[:]
                        )


# ---------------------------------------------------------------------------
# Host-side input prep
# ---------------------------------------------------------------------------


def _perm():
    """Head-dim permutation: interleaved (even,odd) -> [evens; odds]."""
    return np.concatenate([np.arange(0, D, 2), np.arange(1, D, 2)])


def prep_core_inputs(x_b, W_attn, b_attn, W_proj, heads, T=T, C=C, TC=512):
    """Build the per-core input map (numpy) for one (batch, head-group)."""
    import ml_dtypes

    bf16 = ml_dtypes.bfloat16
    perm = _perm()
    HPCl = len(heads)
    VC = HPCl * D
    KBC = TC // P

    Wq = W_attn[:, 0:C].reshape(C, H, D)
    Wk = W_attn[:, C : 2 * C].reshape(C, H, D)
    Wv = W_attn[:, 2 * C : 3 * C].reshape(C, H, D)
    bq = b_attn[0:C].reshape(H, D)
    bk = b_attn[C : 2 * C].reshape(H, D)
    bv = b_attn[2 * C : 3 * C].reshape(H, D)

    Wq_c = np.concatenate([Wq[:, h][:, perm] for h in heads], axis=1)  # [C, VC]
    Wk_c = np.concatenate([Wk[:, h][:, perm] for h in heads], axis=1)
    Wv_c = np.concatenate([Wv[:, h] for h in heads], axis=1)
    Wqk_c = np.concatenate([Wq_c, Wk_c], axis=1).astype(bf16)  # [C, 2VC]

    bqk = np.stack(
        [bq[h][perm] for h in heads] + [bk[h][perm] for h in heads], axis=1
    ).astype(np.float32)  # [128, 2*HPC]
    bv_c = np.concatenate([bv[h] for h in heads]).reshape(1, VC).astype(np.float32)

    inv = ROPE_BASE ** (-np.arange(0, D, 2) / D)  # [64]
    ang = np.arange(T)[None, :] * inv[:, None]  # [64, T]
    cos, sin = np.cos(ang).astype(np.float32), np.sin(ang).astype(np.float32)
    trigA = np.concatenate([cos, cos], axis=0)  # [128, T]
    trigB = np.concatenate([-sin, sin], axis=0)

    # triangle mask for diagonal [P, P] sub-blocks: allow p <= f
    pp = np.arange(P)[:, None]
    ff = np.arange(P)[None, :]
    tri = np.where(pp <= ff, 0.0, NEG).astype(np.float32)

    Wp_rows = np.concatenate(
        [W_proj[h * D : (h + 1) * D] for h in heads], axis=0
    ).astype(bf16)  # [VC, C]

    return {
        "xT": np.ascontiguousarray(x_b.T).astype(bf16),
        "Wqk": np.ascontiguousarray(Wqk_c),
        "Wv": np.ascontiguousarray(Wv_c.astype(bf16)),
        "bqk": np.ascontiguousarray(bqk),
        "bv": bv_c,
        "trigA": trigA,
        "trigB": trigB,
        "tri": tri,
        "Wp": np.ascontiguousarray(Wp_rows),
        "onesr": np.ones((1, P), dtype=np.float32),
        "onesc": np.ones((P, 1), dtype=np.float32),
    }


def make_in_maps(x, W_attn, b_attn, W_proj):
    in_maps = []
    for c in range(NCORES):
        b = c // GROUPS
        g = c % GROUPS
        heads = list(range(g * HPC, (g + 1) * HPC))
        in_maps.append(prep_core_inputs(x[b], W_attn, b_attn, W_proj, heads))
    return in_maps


def kernel(x, W_attn, b_attn, W_proj, b_proj):
    from concourse.bass_utils import run_bass_kernel_spmd

    if "nc" not in _CACHE:
        _CACHE["nc"] = build_nc()
    nc = _CACHE["nc"]

    x = np.asarray(x, dtype=np.float32)
    W_attn = np.asarray(W_attn, dtype=np.float32)
    b_attn = np.asarray(b_attn, dtype=np.float32)
    W_proj = np.asarray(W_proj, dtype=np.float32)
    b_proj = np.asarray(b_proj, dtype=np.float32)

    in_maps = make_in_maps(x, W_attn, b_attn, W_proj)
    res = run_bass_kernel_spmd(nc, in_maps, list(range(NCORES)))

    out = np.empty((B, T, C), dtype=np.float32)
    for b in range(B):
        acc = res.results[b * GROUPS]["out"].astype(np.float32).copy()
        for g in range(1, GROUPS):
            acc += res.results[b * GROUPS + g]["out"]
        out[b] = acc + b_proj[None, :]
    return out



# revision 37
# speedup vs baseline: 1.0026x; 1.0026x over previous
"""Multi-head attention (B=2, T=2048, C=2048, H=16, causal, interleaved RoPE)
as a Bass/Tile kernel on 8 Trainium2 NeuronCores.

Sharding: core c handles batch b = c // 4 and heads 4*(c % 4) .. 4*(c % 4)+4.
Each core computes QKV for its heads, RoPE, causal attention, and the partial
output projection (row-parallel W_proj). Host sums the 4 partials per batch
and adds b_proj.

Device layouts (per core):
  - q, k are produced transposed [D=128(part), T] straight out of the QKV
    matmul (lhsT = W block, rhs = x^T).  The head dim is host-permuted to
    [even dims; odd dims] so interleaved RoPE is pure within-half DVE math
    (plus one small intra-SBUF DMA for the half swap).
  - v is produced natural [T(part), D] (lhsT = x^T block, rhs = W_v).
  - scores are computed transposed [Tk(part), Tq] so exp(scores)^T directly
    feeds the PV matmul as the moving operand.  Softmax denominators: DVE
    accumulates per-(partition, column) prob sums across k-blocks (hidden
    under PE), then per head PE reduces them to one row (ones-column
    matmul), DVE takes the reciprocal, PE broadcasts it back to all 128
    partitions (ones-row matmul), and one DVE multiply normalizes the PV
    accumulator on eviction.  The normalization tail is emitted one head
    late so PE is never parked on DVE.
  - all input DMAs are single multi-dim-AP transfers (one per tensor /
    xt chunk) spread across the SP and Activation HWDGE queues; output
    partials are written as bf16, one DMA per 128-token block.
Dtypes: all matmuls bf16 (except fp32r ones-reductions/broadcasts);
accumulation fp32; output partials bf16, summed on host in fp32.
"""

import math

import numpy as np

P = 128  # partitions
B, T, C, H = 2, 2048, 2048, 16
D = C // H  # 128
NCORES = 8
GROUPS = 4  # head-groups per batch
HPC = H // GROUPS  # heads per core = 4
ROPE_BASE = 10000.0
NEG = -1e9

_CACHE = {}


FULL_PARTS = ("dma", "p1", "p2", "p3", "outdma")


def build_nc(T=T, C=C, HPC=HPC, TCX=512, TC=512, reps=1, parts=FULL_PARTS):
    """Build + compile the per-core Bass program (SPMD: same NEFF, 8 cores).

    reps > 1 replicates the whole computation on-device (for benchmarking:
    dispatch overhead cancels between reps=1 and reps=k timings).
    parts subsets the body for timing attribution probes (default: full
    kernel; correctness requires the default).
    """
    import concourse.bacc as bacc
    import concourse.mybir as mybir
    import concourse.tile as tile

    dt = mybir.dt
    Act = mybir.ActivationFunctionType
    CS = C // P  # contraction slabs
    TB = T // P  # token blocks
    VC = HPC * D  # v columns per core (= 512 at full size)
    KBC = TC // P  # k-blocks per Tq chunk
    scale = 1.0 / math.sqrt(D)

    nc = bacc.Bacc("TRN2", target_bir_lowering=False, debug=False)
    with tile.TileContext(nc) as tc:
        with tc.tile_pool(name="dram", bufs=1, space="DRAM") as dram:

            def din(name, shape, dtype):
                return dram.tile(
                    shape, dtype, kind="ExternalInput", name=name, uniquify=False
                )

            xT = din("xT", [C, T], dt.bfloat16)  # x[b].T
            Wqk = din("Wqk", [C, 2 * VC], dt.bfloat16)  # [q|k], permuted
            Wv = din("Wv", [C, VC], dt.bfloat16)
            bqk = din("bqk", [P, 2 * HPC], dt.float32)  # per-dim bias cols (q,k)
            bv = din("bv", [1, VC], dt.float32r)
            trigA = din("trigA", [P, T], dt.float32)  # [cos; cos]
            trigB = din("trigB", [P, T], dt.float32)  # [-sin; sin]
            tri = din("tri", [P, P], dt.float32)  # causal triangle mask
            Wp = din("Wp", [VC, C], dt.bfloat16)  # W_proj rows for this core
            onesr = din("onesr", [1, P], dt.float32r)
            onesc = din("onesc", [P, 1], dt.float32r)
            out = dram.tile(
                [T, C], dt.bfloat16, kind="ExternalOutput", name="out", uniquify=False
            )

            xT_r = xT.rearrange("(s p) t -> p s t", p=P)
            Wqk_r = Wqk.rearrange("(s p) n -> p s n", p=P)
            Wv_r = Wv.rearrange("(s p) n -> p s n", p=P)
            Wp_r = Wp.rearrange("(s p) n -> p s n", p=P)

            if "hoist" in parts:
                # DIAGNOSTIC ONLY: load all weights once, outside the rep
                # loop, to size the per-rep weight-reload bubble.
                with tc.tile_pool(name="hoistw", bufs=1) as hw_pool:
                    pre = {
                        nm: hw_pool.tile(shape, dty, name="hoist_" + nm)
                        for nm, shape, dty in (
                            ("w_sb", [P, CS, 2 * VC], dt.bfloat16),
                            ("wv_sb", [P, CS, VC], dt.bfloat16),
                            ("trigA_sb", [P, T], dt.float32),
                            ("trigB_sb", [P, T], dt.float32),
                            ("bqk_sb", [P, 2 * HPC], dt.float32),
                            ("bv_sb", [1, VC], dt.float32r),
                            ("wp_sb", [P, HPC, C], dt.bfloat16),
                        )
                    }
                    nc.scalar.dma_start(out=pre["w_sb"][:], in_=Wqk_r[:])
                    nc.scalar.dma_start(out=pre["wv_sb"][:], in_=Wv_r[:])
                    nc.scalar.dma_start(out=pre["trigA_sb"], in_=trigA[:])
                    nc.scalar.dma_start(out=pre["trigB_sb"], in_=trigB[:])
                    nc.scalar.dma_start(out=pre["bqk_sb"], in_=bqk[:])
                    nc.scalar.dma_start(out=pre["bv_sb"], in_=bv[:])
                    nc.scalar.dma_start(out=pre["wp_sb"], in_=Wp_r)
                    for rep in range(reps):
                        _emit_body(
                            nc, tc, dt, Act, rep,
                            xT_r, Wqk_r, Wv_r, Wp_r, bqk, bv, trigA, trigB,
                            tri, onesr, onesc, out,
                            T, C, HPC, TCX, TC, CS, TB, VC, KBC, scale, parts,
                            pre=pre,
                        )
            else:
                for rep in range(reps):
                    _emit_body(
                        nc, tc, dt, Act, rep,
                        xT_r, Wqk_r, Wv_r, Wp_r, bqk, bv, trigA, trigB, tri,
                        onesr, onesc, out,
                        T, C, HPC, TCX, TC, CS, TB, VC, KBC, scale, parts,
                    )
    nc.compile()
    return nc


def _emit_body(
    nc, tc, dt, Act, rep,
    xT_r, Wqk_r, Wv_r, Wp_r, bqk, bv, trigA, trigB, tri, onesr, onesc, out,
    T, C, HPC, TCX, TC, CS, TB, VC, KBC, scale, parts=FULL_PARTS, pre=None,
):
    import concourse.bass_isa as bass_isa
    HD = D // 2
    dma = "dma" in parts
    sfx = f"_{rep}"
    with tc.tile_pool(name="persist" + sfx, bufs=1) as persist:
        qrot = persist.tile([P, HPC, T], dt.bfloat16)
        krot = persist.tile([P, HPC, T], dt.bfloat16)
        v_sb = persist.tile([P, TB, VC], dt.bfloat16)
        ones_row = persist.tile([1, P], dt.float32r)
        tri_sb = persist.tile([P, P], dt.float32)
        ones_col = persist.tile([P, 1], dt.float32r)
        ones_bcr = persist.tile([1, P], dt.float32r)
        nc.sync.dma_start(out=ones_col, in_=onesc[:])
        nc.sync.dma_start(out=ones_bcr, in_=onesr[:])
        if dma:
            nc.sync.dma_start(out=ones_row, in_=onesr[:])
            nc.sync.dma_start(out=tri_sb, in_=tri[:])
        else:
            nc.gpsimd.memset(ones_row[:], 1.0)
            nc.gpsimd.memset(tri_sb[:], 0.0)

        # ---------------- Phase 1: QKV + RoPE ----------------
        with tc.tile_pool(name="p1w" + sfx, bufs=1) as p1w, tc.tile_pool(
            name="p1xt" + sfx, bufs=(2 if pre is not None else 3)
        ) as p1xt, tc.tile_pool(name="p1st" + sfx, bufs=2) as p1st, tc.tile_pool(
            name="p1t1" + sfx, bufs=3
        ) as p1t1, tc.tile_pool(
            name="p1ps" + sfx, bufs=6, space="PSUM"
        ) as p1ps:
            # DMA emission ordered by first-need time: xt0 (t=0), q/k weights
            # (t=0), trig+biases (first rope, ~10us), v weights (~40us),
            # xt1 (~47us).
            xt_pre = []
            xt_c = p1xt.tile([P, CS, TCX], dt.bfloat16, tag="xt_sb")
            if dma:
                nc.sync.dma_start(out=xt_c[:], in_=xT_r[:, :, 0:TCX])
            else:
                nc.gpsimd.memset(xt_c[:], 0.0)
            xt_pre.append(xt_c)
            if pre is not None:
                w_sb = pre["w_sb"]
                trigA_sb = pre["trigA_sb"]
                trigB_sb = pre["trigB_sb"]
                bqk_sb = pre["bqk_sb"]
                bv_sb = pre["bv_sb"]
                wv_sb = pre["wv_sb"]
            else:
                w_sb = p1w.tile([P, CS, 2 * VC], dt.bfloat16)
                if dma:
                    nc.scalar.dma_start(out=w_sb[:], in_=Wqk_r[:])
                trigA_sb = p1w.tile([P, T], dt.float32)
                trigB_sb = p1w.tile([P, T], dt.float32)
                bqk_sb = p1w.tile([P, 2 * HPC], dt.float32)
                bv_sb = p1w.tile([1, VC], dt.float32r)
                wv_sb = p1w.tile([P, CS, VC], dt.bfloat16)
                if dma:
                    nc.scalar.dma_start(out=trigA_sb, in_=trigA[:])
                    nc.scalar.dma_start(out=trigB_sb, in_=trigB[:])
                    nc.scalar.dma_start(out=bqk_sb, in_=bqk[:])
                    nc.scalar.dma_start(out=bv_sb, in_=bv[:])
                    nc.scalar.dma_start(out=wv_sb[:], in_=Wv_r[:])
                else:
                    nc.gpsimd.memset(w_sb[:], 0.0)
                    nc.gpsimd.memset(trigA_sb[:], 0.0)
                    nc.gpsimd.memset(trigB_sb[:], 0.0)
                    nc.gpsimd.memset(bqk_sb[:], 0.0)
                    nc.gpsimd.memset(bv_sb[:], 0.0)
                    nc.gpsimd.memset(wv_sb[:], 0.0)
            xt_c = p1xt.tile([P, CS, TCX], dt.bfloat16, tag="xt_sb")
            if dma:
                nc.sync.dma_start(out=xt_c[:], in_=xT_r[:, :, TCX : 2 * TCX])
            else:
                nc.gpsimd.memset(xt_c[:], 0.0)
            xt_pre.append(xt_c)

            NQK = 2 * HPC  # 8 (q|k, head) results per chunk
            for tx in range(T // TCX):
                tsl = slice(tx * TCX, (tx + 1) * TCX)
                if tx < 2:
                    xt_sb = xt_pre[tx]
                else:
                    xt_sb = p1xt.tile([P, CS, TCX], dt.bfloat16, tag="xt_sb")
                    if dma:
                        nc.sync.dma_start(out=xt_sb[:], in_=xT_r[:, :, tsl])
                    else:
                        nc.gpsimd.memset(xt_sb[:], 0.0)
                if "p1" not in parts:
                    continue
                # q, k transposed per head: [D, TCX]; all 8 head results land
                # in one [P, 8, TCX] tile so the RoPE half-swap is 2 DMAs.
                st = p1st.tile([P, NQK, TCX], dt.bfloat16, tag="st")
                sw = p1st.tile([P, NQK, TCX], dt.bfloat16, tag="sw")
                for qk in range(2):
                    for h in range(HPC):
                        idx = qk * HPC + h
                        col = idx * D
                        ps = p1ps.tile([P, TCX], dt.float32)
                        for s in range(CS):
                            nc.tensor.matmul(
                                ps[:],
                                w_sb[:, s, col : col + D],
                                xt_sb[:, s, :],
                                start=(s == 0),
                                stop=(s == CS - 1),
                            )
                        nc.scalar.activation(
                            st[:, idx], ps[:], Act.Identity,
                            bias=bqk_sb[:, idx : idx + 1],
                        )
                # RoPE: rot = st*[cos;cos] + swap(st)*[-sin;sin]
                nc.sync.dma_start(out=sw[0:HD], in_=st[HD : 2 * HD])
                nc.sync.dma_start(out=sw[HD : 2 * HD], in_=st[0:HD])
                for qk in range(2):
                    dest = qrot if qk == 0 else krot
                    for h in range(HPC):
                        idx = qk * HPC + h
                        t1 = p1t1.tile([P, TCX], dt.float32)
                        nc.vector.tensor_mul(t1[:], st[:, idx], trigA_sb[:, tsl])
                        nc.vector.tensor_mul(sw[:, idx], sw[:, idx], trigB_sb[:, tsl])
                        nc.vector.tensor_add(dest[:, h, tsl], t1[:], sw[:, idx])
                # v natural rows
                for tb in range(TCX // P):
                    kb = tx * (TCX // P) + tb
                    ps = p1ps.tile([P, VC], dt.float32)
                    for s in range(CS):
                        nc.tensor.matmul(
                            ps[:],
                            xt_sb[:, s, tb * P : (tb + 1) * P],
                            wv_sb[:, s, :],
                            start=(s == 0),
                            stop=False,
                        )
                    nc.tensor.matmul(
                        ps[:], ones_row[:], bv_sb[:], start=False, stop=True
                    )
                    nc.vector.tensor_copy(out=v_sb[:, kb, :], in_=ps[:])

        # ---------------- Phase 2: causal attention ----------------
        with tc.tile_pool(name="p2a" + sfx, bufs=1) as p2a:
            attnT = p2a.tile([P, HPC, T], dt.bfloat16)
            if pre is not None:
                wp_sb = pre["wp_sb"]
            else:
                wp_sb = p2a.tile([P, HPC, C], dt.bfloat16)
                if dma:
                    nc.sync.dma_start(out=wp_sb, in_=Wp_r)
                elif "p3" in parts:
                    nc.gpsimd.memset(wp_sb[:], 0.0)

            with tc.tile_pool(name="p2probs" + sfx, bufs=6) as p2probs, tc.tile_pool(
                name="p2b" + sfx, bufs=3
            ) as p2b, tc.tile_pool(
                name="p2sc" + sfx, bufs=3, space="PSUM"
            ) as p2sc, tc.tile_pool(
                name="p2acc" + sfx, bufs=2, space="PSUM"
            ) as p2acc, tc.tile_pool(
                name="p2l" + sfx, bufs=2, space="PSUM"
            ) as p2l, tc.tile_pool(
                name="p2bc" + sfx, bufs=1, space="PSUM"
            ) as p2bc:

                def norm_tail(h_, tq_, a_ps_, sum_sb_):
                    # PE-reduce the DVE-accumulated sums to one row, take the
                    # reciprocal, PE-broadcast to all partitions, then
                    # normalize-evict from psum.  Emitted one head late so PE
                    # is never parked on DVE.
                    qsl_ = slice(tq_ * TC, (tq_ + 1) * TC)
                    l_ps = p2l.tile([1, TC], dt.float32)
                    nc.tensor.matmul(
                        l_ps[:], ones_col[:], sum_sb_[:], start=True, stop=True
                    )
                    r_sb = p2b.tile([1, TC], dt.float32r)
                    with nc.allow_low_precision(
                        reason="float32r == fp32 storage; relaxed matmul ok"
                    ):
                        nc.vector.reciprocal(r_sb[:], l_ps[:])
                    rec_bc = p2bc.tile([P, TC], dt.float32)
                    nc.tensor.matmul(
                        rec_bc[:], ones_bcr[:], r_sb[:], start=True, stop=True
                    )
                    rec_sb = p2b.tile([P, TC], dt.float32, tag="rec_sb")
                    nc.scalar.copy(rec_sb[:], rec_bc[:])
                    nc.vector.tensor_mul(
                        attnT[:, h_, qsl_], a_ps_[:], rec_sb[:]
                    )

                pending = None
                for tq in range(T // TC if "p2" in parts else 0):
                    for h in range(HPC):
                        nkb = (tq + 1) * KBC
                        a_ps = p2acc.tile([P, TC], dt.float32)
                        sum_sb = p2b.tile([P, TC], dt.float32r, tag="sum_sb")
                        for kb in range(nkb):
                            # diagonal blocks: columns < col0 are fully masked
                            # -> skip them in scores/exp/sums/PV and mask only
                            # the [P, P] triangle.
                            j = kb - tq * KBC
                            col0 = max(j, 0) * P
                            csl3 = slice(col0, TC)
                            s_ps = p2sc.tile([P, TC], dt.float32)
                            nc.tensor.matmul(
                                s_ps[:, csl3],
                                krot[:, h, kb * P : (kb + 1) * P],
                                qrot[:, h, tq * TC + col0 : (tq + 1) * TC],
                                start=True,
                                stop=True,
                            )
                            if j >= 0:
                                nc.vector.tensor_add(
                                    s_ps[:, col0 : col0 + P],
                                    s_ps[:, col0 : col0 + P],
                                    tri_sb[:],
                                )
                            pt = p2probs.tile([P, TC], dt.bfloat16)
                            nc.scalar.activation(
                                pt[:, csl3], s_ps[:, csl3], Act.Exp, scale=scale
                            )
                            # running per-(partition, column) prob sums on DVE
                            # (hidden under PE); PE reduces them once per head
                            # in norm_tail.
                            with nc.allow_low_precision(
                                reason="float32r == fp32 storage"
                            ):
                                if kb == 0:
                                    nc.vector.tensor_copy(
                                        out=sum_sb[:], in_=pt[:]
                                    )
                                else:
                                    nc.vector.tensor_add(
                                        sum_sb[:, csl3],
                                        sum_sb[:, csl3],
                                        pt[:, csl3],
                                    )
                            nc.tensor.matmul(
                                a_ps[:, csl3],
                                v_sb[:, kb, h * D : (h + 1) * D],
                                pt[:, csl3],
                                start=(kb == 0),
                                stop=(kb == nkb - 1),
                                skip_group_check=True,
                            )
                            if kb == 1 and pending is not None:
                                norm_tail(*pending)
                                pending = None
                        pending = (h, tq, a_ps, sum_sb)
                if pending is not None:
                    norm_tail(*pending)

            # ---------------- Phase 3: output projection ----------------
            with tc.tile_pool(name="p3o" + sfx, bufs=3) as p3o, tc.tile_pool(
                name="p3ps" + sfx, bufs=6, space="PSUM"
            ) as p3ps:
                NCH = 512
                for tb in range(TB if "p3" in parts else 0):
                    o_sb = p3o.tile([P, C], dt.bfloat16)
                    for ncol in range(C // NCH):
                        csl = slice(ncol * NCH, (ncol + 1) * NCH)
                        ps = p3ps.tile([P, NCH], dt.float32)
                        for j in range(HPC):
                            nc.tensor.matmul(
                                ps[:],
                                attnT[:, j, tb * P : (tb + 1) * P],
                                wp_sb[:, j, csl],
                                start=(j == 0),
                                stop=(j == HPC - 1),
                            )
                        if ncol % 2 == 0:
                            nc.scalar.copy(o_sb[:, csl], ps[:])
                        else:
                            nc.vector.tensor_copy(out=o_sb[:, csl], in_=ps[:])
                    if "outdma" in parts:
                        nc.scalar.dma_start(
                            out=out[tb * P : (tb + 1) * P, :], in_=o_sb[:]
                        )


# ---------------------------------------------------------------------------
# Host-side input prep
# ---------------------------------------------------------------------------


def _perm():
    """Head-dim permutation: interleaved (even,odd) -> [evens; odds]."""
    return np.concatenate([np.arange(0, D, 2), np.arange(1, D, 2)])


def prep_core_inputs(x_b, W_attn, b_attn, W_proj, heads, T=T, C=C, TC=512):
    """Build the per-core input map (numpy) for one (batch, head-group)."""
    import ml_dtypes

    bf16 = ml_dtypes.bfloat16
    perm = _perm()
    HPCl = len(heads)
    VC = HPCl * D
    KBC = TC // P

    Wq = W_attn[:, 0:C].reshape(C, H, D)
    Wk = W_attn[:, C : 2 * C].reshape(C, H, D)
    Wv = W_attn[:, 2 * C : 3 * C].reshape(C, H, D)
    bq = b_attn[0:C].reshape(H, D)
    bk = b_attn[C : 2 * C].reshape(H, D)
    bv = b_attn[2 * C : 3 * C].reshape(H, D)

    Wq_c = np.concatenate([Wq[:, h][:, perm] for h in heads], axis=1)  # [C, VC]
    Wk_c = np.concatenate([Wk[:, h][:, perm] for h in heads], axis=1)
    Wv_c = np.concatenate([Wv[:, h] for h in heads], axis=1)
    Wqk_c = np.concatenate([Wq_c, Wk_c], axis=1).astype(bf16)  # [C, 2VC]

    bqk = np.stack(
        [bq[h][perm] for h in heads] + [bk[h][perm] for h in heads], axis=1
    ).astype(np.float32)  # [128, 2*HPC]
    bv_c = np.concatenate([bv[h] for h in heads]).reshape(1, VC).astype(np.float32)

    inv = ROPE_BASE ** (-np.arange(0, D, 2) / D)  # [64]
    ang = np.arange(T)[None, :] * inv[:, None]  # [64, T]
    cos, sin = np.cos(ang).astype(np.float32), np.sin(ang).astype(np.float32)
    trigA = np.concatenate([cos, cos], axis=0)  # [128, T]
    trigB = np.concatenate([-sin, sin], axis=0)

    # triangle mask for diagonal [P, P] sub-blocks: allow p <= f
    pp = np.arange(P)[:, None]
    ff = np.arange(P)[None, :]
    tri = np.where(pp <= ff, 0.0, NEG).astype(np.float32)

    Wp_rows = np.concatenate(
        [W_proj[h * D : (h + 1) * D] for h in heads], axis=0
    ).astype(bf16)  # [VC, C]

    return {
        "xT": np.ascontiguousarray(x_b.T).astype(bf16),
        "Wqk": np.ascontiguousarray(Wqk_c),
        "Wv": np.ascontiguousarray(Wv_c.astype(bf16)),
        "bqk": np.ascontiguousarray(bqk),
        "bv": bv_c,
        "trigA": trigA,
        "trigB": trigB,
        "tri": tri,
        "Wp": np.ascontiguousarray(Wp_rows),
        "onesr": np.ones((1, P), dtype=np.float32),
        "onesc": np.ones((P, 1), dtype=np.float32),
    }


def make_in_maps(x, W_attn, b_attn, W_proj):
    in_maps = []
    for c in range(NCORES):
        b = c // GROUPS
        g = c % GROUPS
        heads = list(range(g * HPC, (g + 1) * HPC))
        in_maps.append(prep_core_inputs(x[b], W_attn, b_attn, W_proj, heads))
    return in_maps


def kernel(x, W_attn, b_attn, W_proj, b_proj):
    from concourse.bass_utils import run_bass_kernel_spmd

    if "nc" not in _CACHE:
        _CACHE["nc"] = build_nc()
    nc = _CACHE["nc"]

    x = np.asarray(x, dtype=np.float32)
    W_attn = np.asarray(W_attn, dtype=np.float32)
    b_attn = np.asarray(b_attn, dtype=np.float32)
    W_proj = np.asarray(W_proj, dtype=np.float32)
    b_proj = np.asarray(b_proj, dtype=np.float32)

    in_maps = make_in_maps(x, W_attn, b_attn, W_proj)
    res = run_bass_kernel_spmd(nc, in_maps, list(range(NCORES)))

    out = np.empty((B, T, C), dtype=np.float32)
    for b in range(B):
        acc = res.results[b * GROUPS]["out"].astype(np.float32).copy()
        for g in range(1, GROUPS):
            acc += res.results[b * GROUPS + g]["out"]
        out[b] = acc + b_proj[None, :]
    return out

